# revision 2
# baseline (speedup 1.0000x reference)
"""MoE feed-forward kernel for Trainium2 (8 NeuronCores, expert-parallel).

Problem (fixed shapes): x [4096, 1024] f32, w_router [8, 1024], w_gate_up
[8, 4096, 1024], w_down [8, 1024, 2048]. Top-2 routing over 8 experts with
renormalized combine weights, SwiGLU FFN per expert, scatter-combine.

Sharding: expert-parallel - core e owns expert e's weights (hosted as
transposed slices), every core computes the full router (fp32) for all
tokens, runs its expert's FFN (bf16, full PE rate), scales rows by its
combine-weight column, and a ReduceScatter sums partial outputs so core e
ends with output rows [512e, 512e+512).  The host concatenates the shards.
"""

import numpy as np

N_TOK, D_MODEL, D_FF, N_EXP = 4096, 1024, 2048, 8
N_CORES = 8
TOK_BLK = N_TOK // N_CORES  # 512 rows of final output per core
CHUNK = 512                 # token chunk processed per inner iteration
N_CHUNK = N_TOK // CHUNK    # 8
KT_D = D_MODEL // 128       # 8  k-tiles over d_model
KT_F = D_FF // 128          # 16 k-tiles over d_ff
MT_GU = (2 * D_FF) // 128   # 32 f-tiles over gate+up rows
MT_G = D_FF // 128          # 16 (gate tiles; up tile m+16 pairs with gate tile m)

_CACHE = {}


def _build_nc():
    import concourse.bacc as bacc
    import concourse.bass as bass
    import concourse.tile as tile
    from concourse import mybir

    f32 = mybir.dt.float32
    bf16 = mybir.dt.bfloat16
    u32 = mybir.dt.uint32
    ts = bass.ts
    X = mybir.AxisListType.X
    ALU = mybir.AluOpType
    ACTF = mybir.ActivationFunctionType

    nc = bacc.Bacc(
        "TRN2",
        target_bir_lowering=False,
        debug=False,
        enable_asserts=False,
        num_devices=N_CORES,
    )

    # ---- kernel I/O ----
    xT = nc.dram_tensor("xT", [D_MODEL, N_TOK], f32, kind="ExternalInput").ap()
    wrT = nc.dram_tensor("wrT", [D_MODEL, N_EXP], f32, kind="ExternalInput").ap()
    wguT = nc.dram_tensor("wguT", [D_MODEL, 2 * D_FF], f32, kind="ExternalInput").ap()
    wdnT = nc.dram_tensor("wdnT", [D_FF, D_MODEL], f32, kind="ExternalInput").ap()
    eid = nc.dram_tensor("eid", [128, 1], f32, kind="ExternalInput").ap()
    y_out = nc.dram_tensor(
        "y_shard", [TOK_BLK, D_MODEL], f32, kind="ExternalOutput"
    ).ap()

    # DRAM-side views with the 128-partition k-tile split made explicit.
    xT_v = xT.rearrange("(k p) (c t) -> p k c t", p=128, t=CHUNK)
    wrT_v = wrT.rearrange("(k p) e -> p k e", p=128)
    wguT_v = wguT.rearrange("(k p) f -> p k f", p=128)
    wdnT_v = wdnT.rearrange("(k p) d -> p k d", p=128)

    with tile.TileContext(nc) as tc:
        with (
            tc.tile_pool(name="const", bufs=1) as const_pool,
            tc.tile_pool(name="xchunk", bufs=2) as xpool,
            tc.tile_pool(name="xbf", bufs=2) as xbpool,
            tc.tile_pool(name="hid", bufs=2) as hpool,
            tc.tile_pool(name="small", bufs=4) as spool,
            tc.tile_pool(name="yout", bufs=3) as ypool,
            tc.tile_pool(name="pr", bufs=2, space="PSUM") as prpool,
            tc.tile_pool(name="pg", bufs=2, space="PSUM") as pgpool,
            tc.tile_pool(name="pu", bufs=2, space="PSUM") as pupool,
            tc.tile_pool(name="po", bufs=2, space="PSUM") as popool,
            tc.tile_pool(name="dram", bufs=1, space="DRAM") as dpool,
        ):
            # ---- resident tensors ----
            wgu_sb = const_pool.tile([128, KT_D, 2 * D_FF], bf16)   # 8 MB
            wdn_sb = const_pool.tile([128, KT_F, D_MODEL], bf16)    # 4 MB
            wr_sb = const_pool.tile([128, KT_D, N_EXP], f32)
            eid_sb = const_pool.tile([128, 1], f32)
            c_col = const_pool.tile([128, N_TOK // 128], f32)       # combine col

            nc.gpsimd.dma_start(wgu_sb[:], wguT_v)  # f32 -> bf16 cast in DMA
            nc.gpsimd.dma_start(wdn_sb[:], wdnT_v)
            nc.sync.dma_start(wr_sb[:], wrT_v)
            nc.sync.dma_start(eid_sb[:], eid)

            ybuf = dpool.tile([N_TOK, D_MODEL], f32)
            rs_out = dpool.tile([TOK_BLK, D_MODEL], f32)

            for c in range(N_CHUNK):
                # ---- load x^T chunk (f32 for router, bf16 for FFN) ----
                xc = xpool.tile([128, KT_D, CHUNK], f32)
                nc.sync.dma_start(xc[:], xT_v[:, :, c, :])
                xb = xbpool.tile([128, KT_D, CHUNK], bf16)
                nc.vector.tensor_copy(xb[:], xc[:])

                # ---- router (fp32) for this chunk's 4 token tiles ----
                for t4 in range(CHUNK // 128):
                    tg = (CHUNK // 128) * c + t4  # global token tile id
                    pr = prpool.tile([128, N_EXP], f32)
                    for k in range(KT_D):
                        nc.tensor.matmul(
                            pr[:],
                            lhsT=xc[:, k, ts(t4, 128)],
                            rhs=wr_sb[:, k, :],
                            start=(k == 0),
                            stop=(k == KT_D - 1),
                        )
                    # exp(logits); row max-shift dropped (|logit| < 30, safe)
                    ex = spool.tile([128, N_EXP], f32, tag="ex")
                    nc.scalar.activation(ex[:], pr[:], ACTF.Exp)
                    # top-2 (combine weights don't need the softmax denom:
                    # it cancels in top_p / (p1 + p2))
                    top8 = spool.tile([128, 8], f32, tag="top8")
                    nc.vector.max(top8[:], ex[:])
                    idx8 = spool.tile([128, 8], u32, tag="idx8")
                    nc.vector.max_index(idx8[:], top8[:], ex[:])
                    s12 = spool.tile([128, 1], f32, tag="s12")
                    nc.vector.reduce_sum(s12[:], top8[:, 0:2], axis=X)
                    r12 = spool.tile([128, 1], f32, tag="r12")
                    nc.vector.reciprocal(r12[:], s12[:])
                    idxf = spool.tile([128, 2], f32, tag="idxf")
                    nc.vector.tensor_copy(idxf[:], idx8[:, 0:2])
                    # m_i = (idx_i == e) * top_i, i in {1, 2}
                    m1 = spool.tile([128, 1], f32, tag="m1")
                    nc.vector.scalar_tensor_tensor(
                        m1[:], idxf[:, 0:1], eid_sb[:], top8[:, 0:1],
                        op0=ALU.is_equal, op1=ALU.mult,
                    )
                    m2 = spool.tile([128, 1], f32, tag="m2")
                    nc.vector.scalar_tensor_tensor(
                        m2[:], idxf[:, 1:2], eid_sb[:], top8[:, 1:2],
                        op0=ALU.is_equal, op1=ALU.mult,
                    )
                    # c_col[:, tg] = (m1 + m2) * r12
                    msum = spool.tile([128, 1], f32, tag="msum")
                    nc.vector.tensor_add(msum[:], m1[:], m2[:])
                    nc.vector.tensor_scalar_mul(c_col[:, ts(tg, 1)], msum[:], r12[:])

                # ---- MM1: gate/up projections + SwiGLU (bf16) ----
                hid = hpool.tile([128, KT_F, CHUNK], bf16)
                for m in range(MT_G):
                    pg = pgpool.tile([128, CHUNK], f32)
                    pu = pupool.tile([128, CHUNK], f32)
                    for k in range(KT_D):
                        nc.tensor.matmul(
                            pg[:],
                            lhsT=wgu_sb[:, k, ts(m, 128)],
                            rhs=xb[:, k, :],
                            start=(k == 0),
                            stop=(k == KT_D - 1),
                        )
                    for k in range(KT_D):
                        nc.tensor.matmul(
                            pu[:],
                            lhsT=wgu_sb[:, k, ts(MT_G + m, 128)],
                            rhs=xb[:, k, :],
                            start=(k == 0),
                            stop=(k == KT_D - 1),
                        )
                    silu = spool.tile([128, CHUNK], f32, tag="silu")
                    nc.scalar.activation(silu[:], pu[:], ACTF.Silu)
                    nc.vector.tensor_mul(hid[:, m, :], pg[:], silu[:])

                # ---- MM2: down projection + combine scale ----
                for t4 in range(CHUNK // 128):
                    tg = (CHUNK // 128) * c + t4
                    yt = ypool.tile([128, D_MODEL], f32)
                    for dc in range(D_MODEL // 512):
                        po = popool.tile([128, 512], f32)
                        for k in range(KT_F):
                            nc.tensor.matmul(
                                po[:],
                                lhsT=hid[:, k, ts(t4, 128)],
                                rhs=wdn_sb[:, k, ts(dc, 512)],
                                start=(k == 0),
                                stop=(k == KT_F - 1),
                            )
                        nc.vector.tensor_scalar_mul(
                            yt[:, ts(dc, 512)], po[:], c_col[:, ts(tg, 1)]
                        )
                    nc.sync.dma_start(
                        ybuf[ts(tg, 128), :], yt[:]
                    )

            # ---- combine across experts: ReduceScatter + store shard ----
            nc.gpsimd.collective_compute(
                "ReduceScatter",
                mybir.AluOpType.add,
                replica_groups=[list(range(N_CORES))],
                ins=[ybuf.opt()],
                outs=[rs_out.opt()],
            )
            nc.sync.dma_start(y_out, rs_out[:])

    nc.compile()
    return nc


def _get_nc():
    if "nc" not in _CACHE:
        _CACHE["nc"] = _build_nc()
    return _CACHE["nc"]


def kernel(x, w_router, w_gate_up, w_down):
    from concourse.bass_utils import run_bass_kernel_spmd

    x = np.ascontiguousarray(np.asarray(x, dtype=np.float32))
    w_router = np.ascontiguousarray(np.asarray(w_router, dtype=np.float32))
    w_gate_up = np.asarray(w_gate_up, dtype=np.float32)
    w_down = np.asarray(w_down, dtype=np.float32)

    xT = np.ascontiguousarray(x.T)                    # [1024, 4096]
    wrT = np.ascontiguousarray(w_router.T)            # [1024, 8]

    in_maps = []
    for e in range(N_CORES):
        in_maps.append(
            {
                "xT": xT,
                "wrT": wrT,
                "wguT": np.ascontiguousarray(w_gate_up[e].T),  # [1024, 4096]
                "wdnT": np.ascontiguousarray(w_down[e].T),     # [2048, 1024]
                "eid": np.full((128, 1), float(e), dtype=np.float32),
            }
        )

    nc = _get_nc()
    res = run_bass_kernel_spmd(nc, in_maps, core_ids=list(range(N_CORES)))
    _CACHE["last_results"] = res
    y = np.concatenate([res.results[e]["y_shard"] for e in range(N_CORES)], axis=0)
    return y.astype(np.float32)
